# revision 7
# baseline (speedup 1.0000x reference)
"""Trainium2 Bass kernel for nn_Exp_loss (exploded-logit / exponomial choice loss).

Math: reference computes, per assortment row b (S=128 items):
    xa = x[assortments[b]]; ya = y[assortments[b]] (one-hot, exactly one chosen)
    chosen = <xa, ya>;  s = sum relu(xa - chosen)
    sorted ascending xs_j, T_j = sum_k relu(xa_k - xs_j), mask one-hot at j* = rank(chosen)
    loss_b = log(term1 - inner) - c,  term1 = exp(-s+c)/(S-j*),
    inner = sum_{k<j*} exp(-T_k+c) w_k,  w_k = 1/((S-k-1)(S-k)), c = floor(s) at j*

In DESCENDING-sorted space d_0>=d_1>=..., with P_i = inclusive prefix sum,
TD_i = P_i - (i+1) d_i, wd_i = 1/(i(i+1)) (wd_0 = 0), i* = #{k: xa_k > chosen},
the mask-free telescoped form (using sum_{i<=i*} wd_i = 1 - 1/(i*+1)) is EXACT:
    arg = 1 - sum_{i=0}^{S-1} wd_i * exp(min(s - TD_i, 0));  loss_b = log(arg) - s
Since wd_i decays ~1/i^2 and exp(s - TD_i) decays below the chosen item, the sum
truncated at rank KT=8 with the exact telescoped correction for ranks in [KT, i*]
    arg ~= 1 - sum_{i<KT} wd_i exp(min(s - TD_i,0)) - relu(1/KT - 1/(i*+1))
has measured total-loss rel err 3.5e-4 on the reference inputs (gate 2e-2).
Only the top-8 values per row are needed: a single max8, no match_replace.
log(arg) is computed on the DVE from the float bits (exponent) plus a cubic
mantissa polynomial (max err 9.3e-4 absolute, ~1e-5 on the total), so the ACT
engine only ever runs Exp: no second activation-table load anywhere.

Distribution: pure data parallel. Assortment rows are sharded 256/core across
8 cores; x is sharded per row by the host gather (the hint's "shard x/y by item
id" + per-assortment gather), y's one-hot content is shipped as the per-row
chosen value (a pure host-side index extraction + x-gather). Each core computes
its 2x128 rows fully on device and returns 128x2 per-row contributions
ln(arg)-s; the host does the final all-reduce mean.
"""

from contextlib import ExitStack

import numpy as np

import concourse.bass as bass
import concourse.bacc as bacc
import concourse.mybir as mybir
from concourse import tile
from concourse.bass_utils import run_bass_kernel_spmd

B, S = 2048, 128
N = B * S
N_CORES = 8
ROWS_PER_CORE = B // N_CORES          # 256
TILES_PER_CORE = ROWS_PER_CORE // 128  # 2
P = 128
KT = 8                                # top-KT ranks kept exactly
LNWD0 = -1.0e4                        # stands in for ln(wd_0) = -inf
LN2 = 0.6931472
# ln(mantissa) cubic on [1,2): ((CB3*m + CB2)*m + CB1)*m + CB0, max err 9.3e-4
CB3, CB2, CB1, CB0 = 0.10668444, -0.71358749, 2.0868737, -1.47904536
# input layout per partition: [x_t0 (S) | x_t1 (S) | chosen (2) | lnwd (KT) | ip1 (KT)]
OFF_CH = TILES_PER_CORE * S
OFF_LNWD = OFF_CH + TILES_PER_CORE
OFF_IP1 = OFF_LNWD + KT
W_IN = OFF_IP1 + KT

F32 = mybir.dt.float32
U32 = mybir.dt.uint32
Alu = mybir.AluOpType
Act = mybir.ActivationFunctionType


def build_program():
    nc = bacc.Bacc()

    xin_d = nc.dram_tensor("xin", [P, W_IN], F32, kind="ExternalInput")
    out_d = nc.dram_tensor("contrib", [P, TILES_PER_CORE], F32, kind="ExternalOutput")

    with tile.TileContext(nc) as tc, ExitStack() as ctx:
        io = ctx.enter_context(tc.tile_pool(name="io", bufs=1))
        big = ctx.enter_context(tc.tile_pool(name="big", bufs=3))
        med = ctx.enter_context(tc.tile_pool(name="med", bufs=14))
        cols = ctx.enter_context(tc.tile_pool(name="cols", bufs=28))

        xin = io.tile([P, W_IN], F32)
        nc.sync.dma_start(xin[:], xin_d[:])
        lnwd = xin[:, OFF_LNWD:OFF_LNWD + KT]
        ip1 = xin[:, OFF_IP1:OFF_IP1 + KT]

        scol = cols.tile([P, TILES_PER_CORE], F32, tag="scol")
        za = cols.tile([P, TILES_PER_CORE], F32, tag="za")
        ngt2 = cols.tile([P, TILES_PER_CORE], F32, tag="ngt2")
        sum_e2 = cols.tile([P, TILES_PER_CORE], F32, tag="sum_e2")
        zeros = io.tile([P, S], F32)
        nc.vector.memset(zeros[:], 0.0)

        for t in range(TILES_PER_CORE):
            xa = xin[:, t * S:(t + 1) * S]
            chosen = xin[:, OFF_CH + t:OFF_CH + t + 1]
            s_col = scol[:, t:t + 1]

            # s = sum relu(xa - chosen); ngt = #{xa > chosen} = i*
            # (tensor_scalar's accum combines with op1, so the relu-sum must be
            # a scalar_tensor_tensor, whose accum_out is a plain sum)
            sjunk = big.tile([P, S], F32, tag="sjunk")
            nc.vector.scalar_tensor_tensor(
                out=sjunk[:], in0=xa, scalar=chosen, in1=zeros[:],
                op0=Alu.subtract, op1=Alu.max, accum_out=s_col,
            )
            gjunk = big.tile([P, S], F32, tag="gjunk")
            nc.vector.tensor_scalar(
                out=gjunk[:], in0=xa, scalar1=chosen, scalar2=0.0,
                op0=Alu.is_gt, op1=Alu.add, accum_out=ngt2[:, t:t + 1],
            )

            # top-8 of each row, descending
            d = med.tile([P, KT], F32, tag="d")
            nc.vector.max(out=d[:, 0:8], in_=xa)

            # P_incl; TD = P_incl - (i+1) d; m2 = max(TD, s) - lnwd
            ps = med.tile([P, KT], F32, tag="ps")
            nc.vector.tensor_tensor_scan(
                out=ps[:], data0=d[:], data1=d[:], initial=0.0,
                op0=Alu.add, op1=Alu.bypass,
            )
            w1 = med.tile([P, KT], F32, tag="w1")
            nc.vector.tensor_tensor(out=w1[:], in0=d[:], in1=ip1, op=Alu.mult)
            td = med.tile([P, KT], F32, tag="td")
            nc.vector.tensor_tensor(out=td[:], in0=ps[:], in1=w1[:], op=Alu.subtract)
            m2 = med.tile([P, KT], F32, tag="m2")
            nc.vector.scalar_tensor_tensor(
                out=m2[:], in0=td[:], scalar=s_col, in1=lnwd,
                op0=Alu.max, op1=Alu.subtract,
            )
            # e = exp(-m2 + s) = wd * exp(min(s - TD, 0)); sum_e = sum e
            e = med.tile([P, KT], F32, tag="e")
            nc.scalar.activation(
                out=e[:], in_=m2[:], func=Act.Exp, bias=s_col, scale=-1.0,
                accum_out=sum_e2[:, t:t + 1])

        # correction relu(1/KT - 1/(i*+1)) for both tiles at once (does not
        # depend on the exps, so it runs during the tile-1 sort window)
        cnt = cols.tile([P, TILES_PER_CORE], F32, tag="cnt")
        nc.vector.tensor_scalar(
            out=cnt[:], in0=ngt2[:], scalar1=1.0, scalar2=None, op0=Alu.add)
        r1 = cols.tile([P, TILES_PER_CORE], F32, tag="r1")
        nc.vector.reciprocal(out=r1[:], in_=cnt[:])
        ru = cols.tile([P, TILES_PER_CORE], F32, tag="ru")
        nc.vector.tensor_scalar(
            out=ru[:], in0=r1[:], scalar1=-1.0, scalar2=1.0 / KT,
            op0=Alu.mult, op1=Alu.add)
        relu_u = cols.tile([P, TILES_PER_CORE], F32, tag="relu_u")
        nc.vector.tensor_scalar(
            out=relu_u[:], in0=ru[:], scalar1=0.0, scalar2=None, op0=Alu.max)

        # z = arg = 1 - sum_e - relu_u
        tcol = cols.tile([P, TILES_PER_CORE], F32, tag="tcol")
        nc.vector.tensor_tensor(
            out=tcol[:], in0=relu_u[:], in1=sum_e2[:], op=Alu.add)
        nc.vector.tensor_scalar(
            out=za[:], in0=tcol[:], scalar1=-1.0, scalar2=1.0,
            op0=Alu.mult, op1=Alu.add)

        # ln(z) from float bits: LN2*(e_bits-127) + cubic(mantissa), all DVE:
        #   contrib = ln(z) - s = cubic_hi(mf) + (LN2*e_bits - s) + (CB0-127*LN2)
        zu = za[:].bitcast(U32)
        ei = cols.tile([P, TILES_PER_CORE], U32, tag="ei")
        nc.vector.tensor_scalar(
            out=ei[:], in0=zu, scalar1=23, scalar2=None,
            op0=Alu.logical_shift_right)
        mi = cols.tile([P, TILES_PER_CORE], U32, tag="mi")
        nc.vector.tensor_scalar(
            out=mi[:], in0=zu, scalar1=0x7FFFFF, scalar2=0x3F800000,
            op0=Alu.bitwise_and, op1=Alu.bitwise_or)
        mf = mi[:].bitcast(F32)
        ef = cols.tile([P, TILES_PER_CORE], F32, tag="ef")
        nc.vector.tensor_copy(out=ef[:], in_=ei[:])
        base2 = cols.tile([P, TILES_PER_CORE], F32, tag="base2")
        nc.vector.scalar_tensor_tensor(
            out=base2[:], in0=ef[:], scalar=LN2, in1=scol[:],
            op0=Alu.mult, op1=Alu.subtract)
        h1 = cols.tile([P, TILES_PER_CORE], F32, tag="h1")
        nc.vector.tensor_scalar(
            out=h1[:], in0=mf, scalar1=CB3, scalar2=CB2, op0=Alu.mult, op1=Alu.add)
        g1 = cols.tile([P, TILES_PER_CORE], F32, tag="g1")
        nc.vector.tensor_tensor(out=g1[:], in0=h1[:], in1=mf, op=Alu.mult)
        g2 = cols.tile([P, TILES_PER_CORE], F32, tag="g2")
        nc.vector.scalar_tensor_tensor(
            out=g2[:], in0=g1[:], scalar=CB1, in1=mf,
            op0=Alu.add, op1=Alu.mult)
        contrib = cols.tile([P, TILES_PER_CORE], F32, tag="contrib")
        nc.vector.scalar_tensor_tensor(
            out=contrib[:], in0=g2[:], scalar=CB0 - 127.0 * LN2, in1=base2[:],
            op0=Alu.add, op1=Alu.add)
        nc.sync.dma_start(out_d[:], contrib[:])

    nc.compile()
    return nc


def make_inputs(x, y, assortments):
    """Host-side sharding: gathers + index extraction only (no arithmetic)."""
    x = np.ascontiguousarray(np.asarray(x, dtype=np.float32).reshape(N))
    y = np.asarray(y, dtype=np.float32).reshape(N)
    a = np.ascontiguousarray(np.asarray(assortments, dtype=np.int32).reshape(B, S))

    # per-row chosen value: the x at the row's one-hot item (pure index work --
    # assortments partition [0,N), so positions of y's nonzeros map to rows)
    hot = np.flatnonzero(y)                      # item ids with y == 1
    inv = np.empty(N, dtype=np.int64)
    inv[a.reshape(-1).astype(np.int64)] = np.arange(N)
    rows_of_hot = inv[hot] // S                  # the row each hot item lives in
    chosen = np.empty(B, dtype=np.float32)
    chosen[rows_of_hot] = x[hot]

    i = np.arange(KT, dtype=np.float64)
    lnwd = np.full(KT, LNWD0, dtype=np.float32)
    lnwd[1:] = np.log(1.0 / (i[1:] * (i[1:] + 1.0))).astype(np.float32)
    ip1 = (i + 1.0).astype(np.float32)
    consts = np.concatenate([lnwd, ip1])

    in_maps = []
    for c in range(N_CORES):
        rows = a[c * ROWS_PER_CORE:(c + 1) * ROWS_PER_CORE]          # [256, S]
        xa = x[rows]                                                 # [256, S]
        ch = chosen[c * ROWS_PER_CORE:(c + 1) * ROWS_PER_CORE]       # [256]
        xin = np.empty((P, W_IN), dtype=np.float32)
        # partition p, tile t -> row 128*t + p
        xin[:, 0:TILES_PER_CORE * S] = (
            xa.reshape(TILES_PER_CORE, P, S).transpose(1, 0, 2).reshape(P, -1))
        xin[:, OFF_CH:OFF_CH + TILES_PER_CORE] = (
            ch.reshape(TILES_PER_CORE, P).transpose(1, 0))
        xin[:, OFF_LNWD:] = consts[None, :]
        in_maps.append({"xin": np.ascontiguousarray(xin)})
    return in_maps


_PROGRAM_CACHE = {}


def kernel(x, y, assortments, _want_trace=False, _trace_kwargs=None):
    assert np.asarray(x).size == N and np.asarray(assortments).shape == (B, S)
    in_maps = make_inputs(x, y, assortments)
    if "nc" not in _PROGRAM_CACHE:
        _PROGRAM_CACHE["nc"] = build_program()
    nc = _PROGRAM_CACHE["nc"]
    res = run_bass_kernel_spmd(
        nc, in_maps, core_ids=list(range(N_CORES)),
        trace=_want_trace, **(_trace_kwargs or {})
    )
    total = np.float64(0.0)
    for c in range(N_CORES):
        total += np.asarray(res.results[c]["contrib"]).reshape(-1).sum(dtype=np.float64)
    out = np.float32(-total / np.float64(B))
    if _want_trace:
        return out, res
    return out
